# revision 15
# baseline (speedup 1.0000x reference)
"""Multi-head attention (RoPE + pos_bias + mask) Trainium2 Bass kernel, v2.

Sharding: data-parallel over batch x tensor-parallel over heads.
Core c handles batch c//4, heads 4*(c%4)..4*(c%4)+3 as two head-pairs.
Host sums the 4 per-core o_proj partials per batch and adds b_o.

Design (from baseline trace analysis):
 - Baseline was PE-bound with the HAM clock gate stuck at K=4/8 (1.2 GHz)
   through attention: serialized half-array matmuls never trip the warm
   threshold.  v2 packs a head-pair's logits as two concurrent row-tiles
   (tile_position from base_partition 0/64), a pair's PV as two
   concurrent col-tiles, and all 4 softmax-denominator 1-col matmuls as
   col-tiles, roughly halving PE busy-time per step; the pipeline is
   then ACT(exp)-paced and flows even if the PE stays cold.
 - pos_bias+mask applied MULTIPLICATIVELY: host precomputes
   eb = exp(pos_bias/sqrt(hd)) * mask in bf16; device does exp(logits)
   on ACT then one all-bf16 SBUF multiply (DVE 2x mode / GPSIMD),
   instead of an fp32 PSUM bias-add.  Masked entries are exactly 0.
 - RoPE rotate-half done as an SBUF->SBUF DMA partition swap of
   u = pq * s1 (sign folded into s1), so rope costs three full-width
   DVE ops + idle DMA bandwidth instead of extra matmuls or strip ops.
 - Denominator reciprocals batched into one [128,512] reciprocal per
   q-block (baseline burned 53us in 16 separate reciprocals).
 - Phase A (qkv+rope) chunk emission interleaved into the first q-block
   kt loop; o_proj partial interleaved per q-block; outputs DMA'd as
   ready.  q/k/v/exp streams in bf16 (same 1 cycle/row on the PE as
   f32r, 2x on DVE, half the DMA bytes).
"""
import numpy as np
import ml_dtypes

import concourse.bass as bass
import concourse.mybir as mybir
import concourse.tile as tile
from concourse.bass_utils import run_bass_kernel_spmd

B, S, D, H, HD = 2, 2048, 1024, 16, 64
NCORES = 8
T = S                # tokens per core (one batch element)
KO = D // 128        # 8 contraction subtiles for the projections
NCH = T // 512       # 4 token chunks
NQC = T // 512       # 4 query blocks
NKT = T // 128       # 16 key tiles

F32 = mybir.dt.float32
F32R = mybir.dt.float32r
BF16 = mybir.dt.bfloat16
AF = mybir.ActivationFunctionType

TRACE = False
LAST_RESULT = None

_waitfix_ctr = [0]


def _split_waits(nc, max_waits=1):
    """walrus accepts only one sync-wait per instruction; move extras onto
    single-wait NoOps on the same engine queue (identical ordering)."""
    total = 0
    for fn in nc.m.functions:
        for bb in fn.blocks:
            out = []
            changed = False
            for ins in bb.instructions:
                si = ins.sync_info
                if si is not None and si.on_wait and len(si.on_wait) > max_waits:
                    waits = list(si.on_wait)
                    for w in waits[:-max_waits]:
                        _waitfix_ctr[0] += 1
                        n = mybir.InstNoOp(
                            name=f"I-waitfix-{_waitfix_ctr[0]}",
                            ins=[], outs=[], engine=ins.engine,
                        )
                        n.sync_info = mybir.SyncInfo(on_wait=[w], on_update=[])
                        out.append(n)
                        total += 1
                    ins.sync_info = mybir.SyncInfo(
                        on_wait=waits[-max_waits:],
                        on_update=list(si.on_update or []),
                    )
                    changed = True
                out.append(ins)
            if changed:
                bb.instructions = out
    return total


def _build():
    nc = bass.Bass()
    xT = nc.declare_dram_parameter("xT", [128, KO, T], BF16, isOutput=False)
    wqk = nc.declare_dram_parameter("wqk", [128, KO, 512], BF16,
                                    isOutput=False)
    wv = nc.declare_dram_parameter("wv", [128, KO, 256], BF16, isOutput=False)
    wo = nc.declare_dram_parameter("wo", [128, 2, D], BF16, isOutput=False)
    cosT = nc.declare_dram_parameter("cosT", [128, T], BF16, isOutput=False)
    sinT = nc.declare_dram_parameter("sinT", [128, T], BF16, isOutput=False)
    seld = nc.declare_dram_parameter("sel", [128, 2, 128], F32R,
                                     isOutput=False)
    # eb[pair, quarter(4), part, ktpos(4), hp, qc, 512]
    ebd = nc.declare_dram_parameter(
        "eb", [2, 4, 128, 4, 2, NQC, 512], BF16, isOutput=False)
    outp = nc.declare_dram_parameter("out", [T, D], F32, isOutput=True)

    with tile.TileContext(nc) as tc:
        with (
            tc.tile_pool(name="const", bufs=1) as cst,
            tc.tile_pool(name="pers", bufs=1) as pers,
            tc.tile_pool(name="ebp", bufs=3) as ebp,
            tc.tile_pool(name="px", bufs=3) as px,
            tc.tile_pool(name="pa", bufs=2) as pa,
            tc.tile_pool(name="pex", bufs=3) as pex,
            tc.tile_pool(name="pbn", bufs=2) as pbn,
            tc.tile_pool(name="pob", bufs=3) as pob,
            tc.tile_pool(name="pap", bufs=2, space="PSUM") as pap,
            tc.tile_pool(name="acc", bufs=1, space="PSUM") as acc,
        ):
            # ---- weights / constants (DMA order matters: SP queue FIFO) ----
            wqk_sb = cst.tile([128, KO, 512], BF16)
            nc.sync.dma_start(wqk_sb[:], wqk[:])
            cos_sb = cst.tile([128, T], BF16)
            nc.sync.dma_start(cos_sb[:], cosT[:])
            sin_sb = cst.tile([128, T], BF16)
            nc.sync.dma_start(sin_sb[:], sinT[:])
            wv_sb = cst.tile([128, KO, 256], BF16)
            nc.sync.dma_start(wv_sb[:], wv[:])

            onesD = cst.tile([128, 1], BF16)
            nc.vector.memset(onesD[:], 1.0)
            sel = cst.tile([128, 2, 128], F32R)
            nc.sync.dma_start(sel[:], seld[:])

            # persistent per-chunk tensors
            qT = [pers.tile([128, 2, 512], BF16, name=f"qT{i}")
                  for i in range(NCH)]
            kT = [pers.tile([128, 2, 512], BF16, name=f"kT{i}")
                  for i in range(NCH)]
            v1 = [pers.tile([128, 4, 256], BF16, name=f"v1{i}")
                  for i in range(NCH)]
            valsT = [pers.tile([128, 2, 512], BF16, name=f"vals{i}")
                     for i in range(NQC)]

            eb_t = {}

            def eb_fetch(g):
                """prefetch eb global-quarter g (= qc*4 + quarter), both
                pairs.  Ring bufs=3/pair -> never queue-blocks x DMAs."""
                if g >= NQC * 4:
                    return
                qc, qtr = divmod(g, 4)
                for pr in range(2):
                    e = ebp.tile([128, 4, 2, 512], BF16, tag=f"eb{pr}",
                                 name=f"eb_{qc}_{pr}_{qtr}")
                    nc.sync.dma_start(e[:], ebd[pr, qtr, :, :, :, qc, :])
                    eb_t[(qc, pr, qtr)] = e

            # x-chunk DMAs interleaved with the first eb quarters
            xc = []
            for ch in range(NCH):
                t = px.tile([128, KO, 512], BF16, tag="xc", name=f"xc{ch}")
                nc.sync.dma_start(t[:], xT[:, :, ch * 512:(ch + 1) * 512])
                xc.append(t)
                if ch < 2:
                    eb_fetch(ch)
            wo_sb = cst.tile([128, 2, D], BF16)
            nc.sync.dma_start(wo_sb[:], wo[:])

            def emit_A_chunk(ch):
                """qkv projection + rope.  rotate_half is a partition swap:
                u = pq*s1 (signed swapped sin), then SBUF->SBUF DMA strip
                copies swap the 32-row halves, then dst = pq*cos + u_sw."""
                cs = ch * 512
                t1b = pa.tile([128, 4, 512], F32, tag="t1")
                ub = pa.tile([128, 4, 512], BF16, tag="ub")
                for pr in range(2):
                    pla = [pap.tile([128, 512], F32, tag="pl", bufs=5,
                                    name=f"pla_{ch}_{pr}_{mi}")
                           for mi in range(2)]
                    for mi in range(2):      # 0 = q, 1 = k
                        blk = (pr * 2 + mi) * 128
                        for ko in range(KO):
                            nc.tensor.matmul(
                                pla[mi][:], wqk_sb[:, ko, blk:blk + 128],
                                xc[ch][:, ko], start=(ko == 0),
                                stop=(ko == KO - 1))
                    for mi in range(2):
                        j = pr * 2 + mi
                        nc.vector.tensor_mul(
                            out=t1b[:, j, :], in0=pla[mi][:],
                            in1=cos_sb[:, cs:cs + 512])
                        nc.vector.tensor_mul(
                            out=ub[:, j, :], in0=pla[mi][:],
                            in1=sin_sb[:, cs:cs + 512])
                usw = pa.tile([128, 4, 512], BF16, tag="usw")
                for b0 in (0, 64):
                    nc.scalar.dma_start(usw[b0:b0 + 32], ub[b0 + 32:b0 + 64])
                    nc.scalar.dma_start(usw[b0 + 32:b0 + 64], ub[b0:b0 + 32])
                for pr in range(2):
                    for mi in range(2):
                        j = pr * 2 + mi
                        dst = qT[ch] if mi == 0 else kT[ch]
                        nc.vector.tensor_add(
                            out=dst[:, pr, :], in0=t1b[:, j, :],
                            in1=usw[:, j, :])
                for tt in range(4):
                    pvw = pap.tile([128, 512], F32, tag="pl", bufs=5,
                                   name=f"pv_{ch}_{tt}")
                    pv = pvw[:, 0:256]
                    for ko in range(KO):
                        nc.tensor.matmul(
                            pv, xc[ch][:, ko, tt * 128:(tt + 1) * 128],
                            wv_sb[:, ko], start=(ko == 0),
                            stop=(ko == KO - 1))
                    nc.vector.tensor_copy(out=v1[ch][:, tt, :], in_=pv)

            # ---------------- attention + o_proj, A interleaved ----------
            # Tails (normalize + o_proj) are emitted AFTER the next q-block's
            # first kt-group so the PE queue never head-of-line blocks on the
            # reciprocal chain (kept HAM re-throttling every qc otherwise).
            SKEW = 2   # kt steps the PV/den matmuls lag behind

            def emit_tail(qc, pvt, den4):
                qs = qc * 512
                rec = pbn.tile([128, 512], F32, tag="rec")
                nc.vector.tensor_copy(out=rec[:], in_=den4[:])
                recf = pbn.tile([128, 512], F32, tag="recf")
                nc.vector.reciprocal(recf[:], rec[:])
                rcast = pbn.tile([128, 512], F32R, tag="rcast")
                nc.vector.tensor_copy(out=rcast[:], in_=recf[:])
                for pr in range(2):
                    bc = pap.tile([128, 512], F32, tag="pl", bufs=5)
                    nc.tensor.matmul(bc[:], sel[:, pr, :], rcast[:],
                                     start=True, stop=True)
                    bcs = pbn.tile([128, 512], F32, tag="bcs")
                    nc.vector.tensor_copy(out=bcs[:], in_=bc[:])
                    nc.vector.tensor_mul(
                        out=valsT[qc][:, pr, :], in0=pvt[pr][:], in1=bcs[:])
                for tt in range(4):
                    for d2 in range(2):
                        po = pap.tile([128, 512], F32, tag="pl", bufs=5)
                        for pr in range(2):
                            nc.tensor.matmul(
                                po[:],
                                valsT[qc][:, pr, tt * 128:(tt + 1) * 128],
                                wo_sb[:, pr, d2 * 512:(d2 + 1) * 512],
                                start=(pr == 0), stop=(pr == 1))
                        ob = pob.tile([128, 512], F32, tag="ob")
                        nc.vector.tensor_copy(out=ob[:], in_=po[:])
                        nc.sync.dma_start(
                            outp[qs + tt * 128:qs + (tt + 1) * 128,
                                 d2 * 512:(d2 + 1) * 512], ob[:])

            prev_tail = None
            for qc in range(NQC):
                pvt = [acc.tile([128, 512], F32, tag=f"pvt{pr}",
                                name=f"pvt_{qc}_{pr}") for pr in range(2)]
                den4 = acc.tile([128, 512], F32, tag="den", name=f"den_{qc}")
                nc.vector.memset(den4[:], 1.0)
                pend = []

                def flush(limit, pvt=pvt, den4=den4, pend=pend):
                    while len(pend) > limit:
                        fkt, fexs = pend.pop(0)
                        fch, tti = fkt // 4, fkt % 4
                        st, sp = (fkt == 0), (fkt == NKT - 1)
                        for fpr in range(2):
                            for hp in range(2):
                                nc.tensor.matmul(
                                    pvt[fpr][hp * 64:(hp + 1) * 64, :],
                                    v1[fch][:, tti,
                                            fpr * 128 + hp * 64:
                                            fpr * 128 + (hp + 1) * 64],
                                    fexs[fpr][hp][:], start=st, stop=sp,
                                    skip_group_check=True)
                        for fpr in range(2):
                            for hp in range(2):
                                hh = 2 * fpr + hp
                                nc.tensor.matmul(
                                    den4[32 * hh:32 * hh + 1, :],
                                    onesD[:], fexs[fpr][hp][:],
                                    start=st, stop=sp,
                                    skip_group_check=True,
                                    tile_position=(0, 32 * hh))

                for kt in range(NKT):
                    if qc == 0 and (kt == 0 or (kt % 4 == 2 and kt < 12)):
                        # chunk g arrives 2 kt-steps before group g needs it
                        emit_A_chunk(kt // 4 + (1 if kt else 0))
                    if kt % 4 == 0:
                        eb_fetch(qc * 4 + kt // 4 + 2)
                    ch, tti = kt // 4, kt % 4
                    exfs = [[None, None], [None, None]]
                    for pr in range(2):
                        for hp in range(2):
                            h0 = hp * 64
                            pl1 = pap.tile([128, 512], F32, tag="pl",
                                           bufs=5)
                            nc.tensor.matmul(
                                pl1[:],
                                kT[ch][h0:h0 + 64, pr,
                                       tti * 128:(tti + 1) * 128],
                                qT[qc][h0:h0 + 64, pr, :],
                                start=True, stop=True)
                            ex1 = pex.tile([128, 512], BF16, tag="ex",
                                           bufs=6)
                            nc.scalar.activation(ex1[:], pl1[:], AF.Exp)
                            exf = pex.tile([128, 512], BF16, tag="exf",
                                           bufs=12)
                            mul_eng = nc.vector if hp == 0 else nc.gpsimd
                            mul_eng.tensor_mul(
                                out=exf[:], in0=ex1[:],
                                in1=eb_t[(qc, pr, kt // 4)][:, tti, hp, :])
                            exfs[pr][hp] = exf
                    pend.append((kt, exfs))
                    flush(SKEW)
                    if kt == 3 and prev_tail is not None:
                        emit_tail(*prev_tail)
                        prev_tail = None
                flush(0)
                prev_tail = (qc, pvt, den4)
            emit_tail(*prev_tail)

    _split_waits(nc)
    return nc


_nc_cache = None


def _get_nc():
    global _nc_cache
    if _nc_cache is None:
        _nc_cache = _build()
    return _nc_cache


def _prep_inputs(x, pos_bias, sinusoidal_pos, mask, W_qkv, W_o):
    scale = np.float32(1.0 / np.sqrt(HD))
    sp = np.asarray(sinusoidal_pos, np.float32)[0, 0]         # [S, HD]

    cos_t = np.cos(sp).T                                      # [HD, S]
    sin_t = np.sin(sp).T
    cos2_np = np.tile(cos_t, (2, 1)).astype(ml_dtypes.bfloat16)
    # s1: signed swapped sin so that u[p]=pq[p]*s1[p] gives, after the
    # 32-row half swap, u_sw[p] = rotate_half(pq)[p] * sin[p]
    s1_64 = np.concatenate([sin_t[HD // 2:], -sin_t[:HD // 2]], axis=0)
    sin2_np = np.tile(s1_64, (2, 1)).astype(ml_dtypes.bfloat16)

    mask01T = (np.asarray(mask)[0, 0].T != 0)                 # [S(k), S(q)]

    Wh = W_qkv.reshape(H, 3 * HD, D)

    in_maps = []
    for c in range(NCORES):
        b, cg = divmod(c, 4)
        hs = [4 * cg + i for i in range(4)]

        xT_np = np.ascontiguousarray(
            x[b].T.reshape(KO, 128, T).transpose(1, 0, 2)
        ).astype(ml_dtypes.bfloat16)                          # [128, KO, T]

        cols = []
        for pr in range(2):
            h0, h1 = hs[2 * pr], hs[2 * pr + 1]
            for mi in range(2):
                lo, hi = mi * HD, (mi + 1) * HD
                s0 = scale if mi == 0 else np.float32(1.0)
                cols.append(np.concatenate(
                    [Wh[h0, lo:hi] * s0, Wh[h1, lo:hi] * s0], axis=0))
        Wqk_c = np.concatenate(cols, axis=0)                  # [512, D]
        wqk_np = np.ascontiguousarray(
            Wqk_c.T.reshape(KO, 128, 512).transpose(1, 0, 2)
        ).astype(ml_dtypes.bfloat16)

        Wv_c = np.concatenate(
            [Wh[h, 2 * HD:] for h in hs], axis=0)             # [256, D]
        wv_np = np.ascontiguousarray(
            Wv_c.T.reshape(KO, 128, 256).transpose(1, 0, 2)
        ).astype(ml_dtypes.bfloat16)

        wo_np = np.empty((128, 2, D), ml_dtypes.bfloat16)
        for pr in range(2):
            h0, h1 = hs[2 * pr], hs[2 * pr + 1]
            wo_np[0:64, pr, :] = W_o[:, h0 * HD:(h0 + 1) * HD].T
            wo_np[64:128, pr, :] = W_o[:, h1 * HD:(h1 + 1) * HD].T

        # eb[pair, quarter, part, ktpos, hp, qc, 512]
        eb_np = np.empty((2, 4, 128, 4, 2, NQC, 512), ml_dtypes.bfloat16)
        for pr in range(2):
            for hp in range(2):
                h = hs[2 * pr + hp]
                ebT = np.exp(pos_bias[0, h].T * scale)
                ebT = np.where(mask01T, ebT, np.float32(0.0))  # [S(k), S(q)]
                r = ebT.reshape(4, 4, 128, NQC, 512)
                eb_np[pr, :, :, :, hp, :, :] = r.transpose(0, 2, 1, 3, 4)

        # sel[p, pr, m] = 1 iff p == 64*pr + 32*(m//64): broadcasts the
        # reciprocal rows (at partitions 32h) down each pair's 64-row halves
        sel_np = np.zeros((128, 2, 128), np.float32)
        for pr in range(2):
            sel_np[64 * pr, pr, 0:64] = 1.0
            sel_np[64 * pr + 32, pr, 64:128] = 1.0
        in_maps.append({
            "xT": xT_np, "wqk": wqk_np, "wv": wv_np, "wo": wo_np,
            "cosT": cos2_np, "sinT": sin2_np, "sel": sel_np,
            "eb": np.ascontiguousarray(eb_np),
        })
    return in_maps


def _ensure_profile_hook():
    import sys
    import types
    try:
        from antenv.axon_hooks import get_axon_ntff_profile_hook  # noqa
        return
    except ImportError:
        pass
    try:
        from trn_agent_boot.trn_boot import _ntff_profile_via_ctypes
        hook = _ntff_profile_via_ctypes("/opt/axon/libaxon_pjrt.so")
        mod = types.ModuleType("antenv.axon_hooks")
        mod.get_axon_ntff_profile_hook = lambda: hook
        mod.set_axon_ntff_profile_hook = lambda h: None
        sys.modules["antenv.axon_hooks"] = mod
    except Exception:
        pass


def kernel(x, pos_bias, sinusoidal_pos, mask, W_qkv, b_qkv, W_o, b_o):
    global LAST_RESULT
    if TRACE:
        _ensure_profile_hook()
    x = np.asarray(x, np.float32)
    pos_bias = np.asarray(pos_bias, np.float32)
    W_qkv = np.asarray(W_qkv, np.float32)
    W_o = np.asarray(W_o, np.float32)
    b_qkv = np.asarray(b_qkv, np.float32)
    assert not np.any(b_qkv), "nonzero b_qkv not supported by this kernel"
    in_maps = _prep_inputs(x, pos_bias, sinusoidal_pos, mask, W_qkv, W_o)
    nc = _get_nc()
    try:
        r = run_bass_kernel_spmd(nc, in_maps, list(range(NCORES)),
                                 trace=TRACE)
    except Exception:
        r = run_bass_kernel_spmd(nc, in_maps, list(range(NCORES)),
                                 trace=TRACE)
    LAST_RESULT = r
    b_o64 = np.asarray(b_o, np.float32).astype(np.float64)
    out = np.empty((B, S, D), np.float32)
    for b in range(B):
        partial = np.zeros((T, D), np.float64)
        for cg in range(4):
            partial += r.results[4 * b + cg]["out"].astype(np.float64)
        out[b] = (partial + b_o64).astype(np.float32)
    return out


# revision 16
# speedup vs baseline: 1.1204x; 1.1204x over previous
"""Multi-head attention (RoPE + pos_bias + mask) Trainium2 Bass kernel, v2.

Sharding: data-parallel over batch x tensor-parallel over heads.
Core c handles batch c//4, heads 4*(c%4)..4*(c%4)+3 as two head-pairs.
Host sums the 4 per-core o_proj partials per batch and adds b_o.

Design (from baseline trace analysis):
 - Baseline was PE-bound with the HAM clock gate stuck at K=4/8 (1.2 GHz)
   through attention: serialized half-array matmuls never trip the warm
   threshold.  v2 packs a head-pair's logits as two concurrent row-tiles
   (tile_position from base_partition 0/64), a pair's PV as two
   concurrent col-tiles, and all 4 softmax-denominator 1-col matmuls as
   col-tiles, roughly halving PE busy-time per step; the pipeline is
   then ACT(exp)-paced and flows even if the PE stays cold.
 - pos_bias+mask applied MULTIPLICATIVELY: host precomputes
   eb = exp(pos_bias/sqrt(hd)) * mask in bf16; device does exp(logits)
   on ACT then one all-bf16 SBUF multiply (DVE 2x mode / GPSIMD),
   instead of an fp32 PSUM bias-add.  Masked entries are exactly 0.
 - RoPE rotate-half done as an SBUF->SBUF DMA partition swap of
   u = pq * s1 (sign folded into s1), so rope costs three full-width
   DVE ops + idle DMA bandwidth instead of extra matmuls or strip ops.
 - Denominator reciprocals batched into one [128,512] reciprocal per
   q-block (baseline burned 53us in 16 separate reciprocals).
 - Phase A (qkv+rope) chunk emission interleaved into the first q-block
   kt loop; o_proj partial interleaved per q-block; outputs DMA'd as
   ready.  q/k/v/exp streams in bf16 (same 1 cycle/row on the PE as
   f32r, 2x on DVE, half the DMA bytes).
"""
import numpy as np
import ml_dtypes

import concourse.bass as bass
import concourse.mybir as mybir
import concourse.tile as tile
from concourse.bass_utils import run_bass_kernel_spmd

B, S, D, H, HD = 2, 2048, 1024, 16, 64
NCORES = 8
T = S                # tokens per core (one batch element)
KO = D // 128        # 8 contraction subtiles for the projections
NCH = T // 512       # 4 token chunks
NQC = T // 512       # 4 query blocks
NKT = T // 128       # 16 key tiles

F32 = mybir.dt.float32
F32R = mybir.dt.float32r
BF16 = mybir.dt.bfloat16
AF = mybir.ActivationFunctionType

TRACE = False
LAST_RESULT = None

_waitfix_ctr = [0]


def _split_waits(nc, max_waits=1):
    """walrus accepts only one sync-wait per instruction; move extras onto
    single-wait NoOps on the same engine queue (identical ordering)."""
    total = 0
    for fn in nc.m.functions:
        for bb in fn.blocks:
            out = []
            changed = False
            for ins in bb.instructions:
                si = ins.sync_info
                if si is not None and si.on_wait and len(si.on_wait) > max_waits:
                    waits = list(si.on_wait)
                    for w in waits[:-max_waits]:
                        _waitfix_ctr[0] += 1
                        n = mybir.InstNoOp(
                            name=f"I-waitfix-{_waitfix_ctr[0]}",
                            ins=[], outs=[], engine=ins.engine,
                        )
                        n.sync_info = mybir.SyncInfo(on_wait=[w], on_update=[])
                        out.append(n)
                        total += 1
                    ins.sync_info = mybir.SyncInfo(
                        on_wait=waits[-max_waits:],
                        on_update=list(si.on_update or []),
                    )
                    changed = True
                out.append(ins)
            if changed:
                bb.instructions = out
    return total


def _build():
    nc = bass.Bass()
    xT = nc.declare_dram_parameter("xT", [128, KO, T], BF16, isOutput=False)
    wqk = nc.declare_dram_parameter("wqk", [128, KO, 512], BF16,
                                    isOutput=False)
    wv = nc.declare_dram_parameter("wv", [128, KO, 256], BF16, isOutput=False)
    wo = nc.declare_dram_parameter("wo", [128, 2, D], BF16, isOutput=False)
    cosT = nc.declare_dram_parameter("cosT", [128, T], BF16, isOutput=False)
    sinT = nc.declare_dram_parameter("sinT", [128, T], BF16, isOutput=False)
    seld = nc.declare_dram_parameter("sel", [128, 2, 128], F32R,
                                     isOutput=False)
    # eb[pair, quarter(4), part, ktpos(4), hp, qc, 512]
    ebd = nc.declare_dram_parameter(
        "eb", [2, 4, 128, 4, 2, NQC, 512], BF16, isOutput=False)
    outp = nc.declare_dram_parameter("out", [T, D], F32, isOutput=True)

    with tile.TileContext(nc) as tc:
        with (
            tc.tile_pool(name="const", bufs=1) as cst,
            tc.tile_pool(name="pers", bufs=1) as pers,
            tc.tile_pool(name="ebp", bufs=3) as ebp,
            tc.tile_pool(name="px", bufs=3) as px,
            tc.tile_pool(name="pa", bufs=2) as pa,
            tc.tile_pool(name="pex", bufs=3) as pex,
            tc.tile_pool(name="pbn", bufs=2) as pbn,
            tc.tile_pool(name="pob", bufs=3) as pob,
            tc.tile_pool(name="pap", bufs=2, space="PSUM") as pap,
            tc.tile_pool(name="acc", bufs=1, space="PSUM") as acc,
        ):
            # ---- weights / constants (DMA order matters: SP queue FIFO) ----
            wqk_sb = cst.tile([128, KO, 512], BF16)
            nc.sync.dma_start(wqk_sb[:], wqk[:])
            cos_sb = cst.tile([128, T], BF16)
            nc.sync.dma_start(cos_sb[:], cosT[:])
            sin_sb = cst.tile([128, T], BF16)
            nc.sync.dma_start(sin_sb[:], sinT[:])
            wv_sb = cst.tile([128, KO, 256], BF16)
            nc.sync.dma_start(wv_sb[:], wv[:])

            onesD = cst.tile([128, 1], BF16)
            nc.vector.memset(onesD[:], 1.0)
            sel = cst.tile([128, 2, 128], F32R)
            nc.sync.dma_start(sel[:], seld[:])

            # persistent per-chunk tensors
            qT = [pers.tile([128, 2, 512], BF16, name=f"qT{i}")
                  for i in range(NCH)]
            kT = [pers.tile([128, 2, 512], BF16, name=f"kT{i}")
                  for i in range(NCH)]
            v1 = [pers.tile([128, 4, 256], BF16, name=f"v1{i}")
                  for i in range(NCH)]
            valsT = [pers.tile([128, 2, 512], BF16, name=f"vals{i}")
                     for i in range(NQC)]

            eb_t = {}

            def eb_fetch(g):
                """prefetch eb global-quarter g (= qc*4 + quarter), both
                pairs.  Ring bufs=3/pair -> never queue-blocks x DMAs."""
                if g >= NQC * 4:
                    return
                qc, qtr = divmod(g, 4)
                for pr in range(2):
                    e = ebp.tile([128, 4, 2, 512], BF16, tag=f"eb{pr}",
                                 name=f"eb_{qc}_{pr}_{qtr}")
                    nc.sync.dma_start(e[:], ebd[pr, qtr, :, :, :, qc, :])
                    eb_t[(qc, pr, qtr)] = e

            # x-chunk DMAs interleaved with the first eb quarters
            xc = []
            for ch in range(NCH):
                t = px.tile([128, KO, 512], BF16, tag="xc", name=f"xc{ch}")
                nc.sync.dma_start(t[:], xT[:, :, ch * 512:(ch + 1) * 512])
                xc.append(t)
                if ch < 2:
                    eb_fetch(ch)
            wo_sb = cst.tile([128, 2, D], BF16)
            nc.sync.dma_start(wo_sb[:], wo[:])

            def emit_A_chunk(ch):
                """qkv projection + rope.  rotate_half is a partition swap:
                u = pq*s1 (signed swapped sin), then SBUF->SBUF DMA strip
                copies swap the 32-row halves, then dst = pq*cos + u_sw."""
                cs = ch * 512
                t1b = pa.tile([128, 4, 512], F32, tag="t1")
                ub = pa.tile([128, 4, 512], BF16, tag="ub")
                for pr in range(2):
                    pla = [pap.tile([128, 512], F32, tag="pl", bufs=5,
                                    name=f"pla_{ch}_{pr}_{mi}")
                           for mi in range(2)]
                    for mi in range(2):      # 0 = q, 1 = k
                        blk = (pr * 2 + mi) * 128
                        for ko in range(KO):
                            nc.tensor.matmul(
                                pla[mi][:], wqk_sb[:, ko, blk:blk + 128],
                                xc[ch][:, ko], start=(ko == 0),
                                stop=(ko == KO - 1))
                    for mi in range(2):
                        j = pr * 2 + mi
                        nc.vector.tensor_mul(
                            out=t1b[:, j, :], in0=pla[mi][:],
                            in1=cos_sb[:, cs:cs + 512])
                        nc.vector.tensor_mul(
                            out=ub[:, j, :], in0=pla[mi][:],
                            in1=sin_sb[:, cs:cs + 512])
                usw = pa.tile([128, 4, 512], BF16, tag="usw")
                for b0 in (0, 64):
                    nc.scalar.dma_start(usw[b0:b0 + 32], ub[b0 + 32:b0 + 64])
                    nc.scalar.dma_start(usw[b0 + 32:b0 + 64], ub[b0:b0 + 32])
                for pr in range(2):
                    for mi in range(2):
                        j = pr * 2 + mi
                        dst = qT[ch] if mi == 0 else kT[ch]
                        nc.vector.tensor_add(
                            out=dst[:, pr, :], in0=t1b[:, j, :],
                            in1=usw[:, j, :])
                for tt in range(4):
                    pvw = pap.tile([128, 512], F32, tag="pl", bufs=5,
                                   name=f"pv_{ch}_{tt}")
                    pv = pvw[:, 0:256]
                    for ko in range(KO):
                        nc.tensor.matmul(
                            pv, xc[ch][:, ko, tt * 128:(tt + 1) * 128],
                            wv_sb[:, ko], start=(ko == 0),
                            stop=(ko == KO - 1))
                    nc.vector.tensor_copy(out=v1[ch][:, tt, :], in_=pv)

            # ---------------- attention + o_proj, A interleaved ----------
            # Tails (normalize + o_proj) are emitted AFTER the next q-block's
            # first kt-group so the PE queue never head-of-line blocks on the
            # reciprocal chain (kept HAM re-throttling every qc otherwise).
            SKEW = 1   # kt steps the PV/den matmuls lag behind

            def emit_tail(qc, pvt, den4):
                qs = qc * 512
                rec = pbn.tile([128, 512], F32, tag="rec")
                nc.vector.tensor_copy(out=rec[:], in_=den4[:])
                recf = pbn.tile([128, 512], F32, tag="recf")
                nc.vector.reciprocal(recf[:], rec[:])
                rcast = pbn.tile([128, 512], F32R, tag="rcast")
                nc.vector.tensor_copy(out=rcast[:], in_=recf[:])
                for pr in range(2):
                    bc = pap.tile([128, 512], F32, tag="pl", bufs=5)
                    nc.tensor.matmul(bc[:], sel[:, pr, :], rcast[:],
                                     start=True, stop=True)
                    bcs = pbn.tile([128, 512], F32, tag="bcs")
                    nc.vector.tensor_copy(out=bcs[:], in_=bc[:])
                    nc.vector.tensor_mul(
                        out=valsT[qc][:, pr, :], in0=pvt[pr][:], in1=bcs[:])
                for tt in range(4):
                    for d2 in range(2):
                        po = pap.tile([128, 512], F32, tag="pl", bufs=5)
                        for pr in range(2):
                            nc.tensor.matmul(
                                po[:],
                                valsT[qc][:, pr, tt * 128:(tt + 1) * 128],
                                wo_sb[:, pr, d2 * 512:(d2 + 1) * 512],
                                start=(pr == 0), stop=(pr == 1))
                        ob = pob.tile([128, 512], F32, tag="ob")
                        nc.vector.tensor_copy(out=ob[:], in_=po[:])
                        nc.sync.dma_start(
                            outp[qs + tt * 128:qs + (tt + 1) * 128,
                                 d2 * 512:(d2 + 1) * 512], ob[:])

            prev_tail = None
            for qc in range(NQC):
                pvt = [acc.tile([128, 512], F32, tag=f"pvt{pr}",
                                name=f"pvt_{qc}_{pr}") for pr in range(2)]
                den4 = acc.tile([128, 512], F32, tag="den", name=f"den_{qc}")
                nc.vector.memset(den4[:], 1.0)
                pend = []

                def flush(limit, pvt=pvt, den4=den4, pend=pend):
                    while len(pend) > limit:
                        fkt, fexs = pend.pop(0)
                        fch, tti = fkt // 4, fkt % 4
                        st, sp = (fkt == 0), (fkt == NKT - 1)
                        for fpr in range(2):
                            for hp in range(2):
                                nc.tensor.matmul(
                                    pvt[fpr][hp * 64:(hp + 1) * 64, :],
                                    v1[fch][:, tti,
                                            fpr * 128 + hp * 64:
                                            fpr * 128 + (hp + 1) * 64],
                                    fexs[fpr][hp][:], start=st, stop=sp,
                                    skip_group_check=True)
                        for fpr in range(2):
                            for hp in range(2):
                                hh = 2 * fpr + hp
                                nc.tensor.matmul(
                                    den4[32 * hh:32 * hh + 1, :],
                                    onesD[:], fexs[fpr][hp][:],
                                    start=st, stop=sp,
                                    skip_group_check=True,
                                    tile_position=(0, 32 * hh))

                for kt in range(NKT):
                    if qc == 0 and (kt == 0 or (kt % 4 == 2 and kt < 12)):
                        # chunk g arrives 2 kt-steps before group g needs it
                        emit_A_chunk(kt // 4 + (1 if kt else 0))
                    if kt % 4 == 0:
                        eb_fetch(qc * 4 + kt // 4 + 2)
                    ch, tti = kt // 4, kt % 4
                    exfs = [[None, None], [None, None]]
                    for pr in range(2):
                        for hp in range(2):
                            h0 = hp * 64
                            pl1 = pap.tile([128, 512], F32, tag="pl",
                                           bufs=5)
                            nc.tensor.matmul(
                                pl1[:],
                                kT[ch][h0:h0 + 64, pr,
                                       tti * 128:(tti + 1) * 128],
                                qT[qc][h0:h0 + 64, pr, :],
                                start=True, stop=True)
                            ex1 = pex.tile([128, 512], BF16, tag="ex",
                                           bufs=6)
                            nc.scalar.activation(ex1[:], pl1[:], AF.Exp)
                            exf = pex.tile([128, 512], BF16, tag="exf",
                                           bufs=8)
                            mul_eng = nc.vector if hp == 0 else nc.gpsimd
                            mul_eng.tensor_mul(
                                out=exf[:], in0=ex1[:],
                                in1=eb_t[(qc, pr, kt // 4)][:, tti, hp, :])
                            exfs[pr][hp] = exf
                    pend.append((kt, exfs))
                    flush(SKEW)
                    if kt == 3 and prev_tail is not None:
                        emit_tail(*prev_tail)
                        prev_tail = None
                flush(0)
                prev_tail = (qc, pvt, den4)
            emit_tail(*prev_tail)

    _split_waits(nc)
    return nc


_nc_cache = None


def _get_nc():
    global _nc_cache
    if _nc_cache is None:
        _nc_cache = _build()
    return _nc_cache


def _prep_inputs(x, pos_bias, sinusoidal_pos, mask, W_qkv, W_o):
    scale = np.float32(1.0 / np.sqrt(HD))
    sp = np.asarray(sinusoidal_pos, np.float32)[0, 0]         # [S, HD]

    cos_t = np.cos(sp).T                                      # [HD, S]
    sin_t = np.sin(sp).T
    cos2_np = np.tile(cos_t, (2, 1)).astype(ml_dtypes.bfloat16)
    # s1: signed swapped sin so that u[p]=pq[p]*s1[p] gives, after the
    # 32-row half swap, u_sw[p] = rotate_half(pq)[p] * sin[p]
    s1_64 = np.concatenate([sin_t[HD // 2:], -sin_t[:HD // 2]], axis=0)
    sin2_np = np.tile(s1_64, (2, 1)).astype(ml_dtypes.bfloat16)

    mask01T = (np.asarray(mask)[0, 0].T != 0)                 # [S(k), S(q)]

    Wh = W_qkv.reshape(H, 3 * HD, D)

    in_maps = []
    for c in range(NCORES):
        b, cg = divmod(c, 4)
        hs = [4 * cg + i for i in range(4)]

        xT_np = np.ascontiguousarray(
            x[b].T.reshape(KO, 128, T).transpose(1, 0, 2)
        ).astype(ml_dtypes.bfloat16)                          # [128, KO, T]

        cols = []
        for pr in range(2):
            h0, h1 = hs[2 * pr], hs[2 * pr + 1]
            for mi in range(2):
                lo, hi = mi * HD, (mi + 1) * HD
                s0 = scale if mi == 0 else np.float32(1.0)
                cols.append(np.concatenate(
                    [Wh[h0, lo:hi] * s0, Wh[h1, lo:hi] * s0], axis=0))
        Wqk_c = np.concatenate(cols, axis=0)                  # [512, D]
        wqk_np = np.ascontiguousarray(
            Wqk_c.T.reshape(KO, 128, 512).transpose(1, 0, 2)
        ).astype(ml_dtypes.bfloat16)

        Wv_c = np.concatenate(
            [Wh[h, 2 * HD:] for h in hs], axis=0)             # [256, D]
        wv_np = np.ascontiguousarray(
            Wv_c.T.reshape(KO, 128, 256).transpose(1, 0, 2)
        ).astype(ml_dtypes.bfloat16)

        wo_np = np.empty((128, 2, D), ml_dtypes.bfloat16)
        for pr in range(2):
            h0, h1 = hs[2 * pr], hs[2 * pr + 1]
            wo_np[0:64, pr, :] = W_o[:, h0 * HD:(h0 + 1) * HD].T
            wo_np[64:128, pr, :] = W_o[:, h1 * HD:(h1 + 1) * HD].T

        # eb[pair, quarter, part, ktpos, hp, qc, 512]
        eb_np = np.empty((2, 4, 128, 4, 2, NQC, 512), ml_dtypes.bfloat16)
        for pr in range(2):
            for hp in range(2):
                h = hs[2 * pr + hp]
                ebT = np.exp(pos_bias[0, h].T * scale)
                ebT = np.where(mask01T, ebT, np.float32(0.0))  # [S(k), S(q)]
                r = ebT.reshape(4, 4, 128, NQC, 512)
                eb_np[pr, :, :, :, hp, :, :] = r.transpose(0, 2, 1, 3, 4)

        # sel[p, pr, m] = 1 iff p == 64*pr + 32*(m//64): broadcasts the
        # reciprocal rows (at partitions 32h) down each pair's 64-row halves
        sel_np = np.zeros((128, 2, 128), np.float32)
        for pr in range(2):
            sel_np[64 * pr, pr, 0:64] = 1.0
            sel_np[64 * pr + 32, pr, 64:128] = 1.0
        in_maps.append({
            "xT": xT_np, "wqk": wqk_np, "wv": wv_np, "wo": wo_np,
            "cosT": cos2_np, "sinT": sin2_np, "sel": sel_np,
            "eb": np.ascontiguousarray(eb_np),
        })
    return in_maps


def _ensure_profile_hook():
    import sys
    import types
    try:
        from antenv.axon_hooks import get_axon_ntff_profile_hook  # noqa
        return
    except ImportError:
        pass
    try:
        from trn_agent_boot.trn_boot import _ntff_profile_via_ctypes
        hook = _ntff_profile_via_ctypes("/opt/axon/libaxon_pjrt.so")
        mod = types.ModuleType("antenv.axon_hooks")
        mod.get_axon_ntff_profile_hook = lambda: hook
        mod.set_axon_ntff_profile_hook = lambda h: None
        sys.modules["antenv.axon_hooks"] = mod
    except Exception:
        pass


def kernel(x, pos_bias, sinusoidal_pos, mask, W_qkv, b_qkv, W_o, b_o):
    global LAST_RESULT
    if TRACE:
        _ensure_profile_hook()
    x = np.asarray(x, np.float32)
    pos_bias = np.asarray(pos_bias, np.float32)
    W_qkv = np.asarray(W_qkv, np.float32)
    W_o = np.asarray(W_o, np.float32)
    b_qkv = np.asarray(b_qkv, np.float32)
    assert not np.any(b_qkv), "nonzero b_qkv not supported by this kernel"
    in_maps = _prep_inputs(x, pos_bias, sinusoidal_pos, mask, W_qkv, W_o)
    nc = _get_nc()
    try:
        r = run_bass_kernel_spmd(nc, in_maps, list(range(NCORES)),
                                 trace=TRACE)
    except Exception:
        r = run_bass_kernel_spmd(nc, in_maps, list(range(NCORES)),
                                 trace=TRACE)
    LAST_RESULT = r
    b_o64 = np.asarray(b_o, np.float32).astype(np.float64)
    out = np.empty((B, S, D), np.float32)
    for b in range(B):
        partial = np.zeros((T, D), np.float64)
        for cg in range(4):
            partial += r.results[4 * b + cg]["out"].astype(np.float64)
        out[b] = (partial + b_o64).astype(np.float32)
    return out


# revision 17
# speedup vs baseline: 1.1389x; 1.0165x over previous
"""Multi-head attention (RoPE + pos_bias + mask) Trainium2 Bass kernel, v2.

Sharding: data-parallel over batch x tensor-parallel over heads.
Core c handles batch c//4, heads 4*(c%4)..4*(c%4)+3 as two head-pairs.
Host sums the 4 per-core o_proj partials per batch and adds b_o.

Design (from baseline trace analysis):
 - Baseline was PE-bound with the HAM clock gate stuck at K=4/8 (1.2 GHz)
   through attention: serialized half-array matmuls never trip the warm
   threshold.  v2 packs a head-pair's logits as two concurrent row-tiles
   (tile_position from base_partition 0/64), a pair's PV as two
   concurrent col-tiles, and all 4 softmax-denominator 1-col matmuls as
   col-tiles, roughly halving PE busy-time per step; the pipeline is
   then ACT(exp)-paced and flows even if the PE stays cold.
 - pos_bias+mask applied MULTIPLICATIVELY: host precomputes
   eb = exp(pos_bias/sqrt(hd)) * mask in bf16; device does exp(logits)
   on ACT then one all-bf16 SBUF multiply (DVE 2x mode / GPSIMD),
   instead of an fp32 PSUM bias-add.  Masked entries are exactly 0.
 - RoPE rotate-half done as an SBUF->SBUF DMA partition swap of
   u = pq * s1 (sign folded into s1), so rope costs three full-width
   DVE ops + idle DMA bandwidth instead of extra matmuls or strip ops.
 - Denominator reciprocals batched into one [128,512] reciprocal per
   q-block (baseline burned 53us in 16 separate reciprocals).
 - Phase A (qkv+rope) chunk emission interleaved into the first q-block
   kt loop; o_proj partial interleaved per q-block; outputs DMA'd as
   ready.  q/k/v/exp streams in bf16 (same 1 cycle/row on the PE as
   f32r, 2x on DVE, half the DMA bytes).
"""
import numpy as np
import ml_dtypes

import concourse.bass as bass
import concourse.mybir as mybir
import concourse.tile as tile
from concourse.bass_utils import run_bass_kernel_spmd

B, S, D, H, HD = 2, 2048, 1024, 16, 64
NCORES = 8
T = S                # tokens per core (one batch element)
KO = D // 128        # 8 contraction subtiles for the projections
NCH = T // 512       # 4 token chunks
NQC = T // 512       # 4 query blocks
NKT = T // 128       # 16 key tiles

F32 = mybir.dt.float32
F32R = mybir.dt.float32r
BF16 = mybir.dt.bfloat16
AF = mybir.ActivationFunctionType

TRACE = False
LAST_RESULT = None

_waitfix_ctr = [0]


def _split_waits(nc, max_waits=1):
    """walrus accepts only one sync-wait per instruction; move extras onto
    single-wait NoOps on the same engine queue (identical ordering)."""
    total = 0
    for fn in nc.m.functions:
        for bb in fn.blocks:
            out = []
            changed = False
            for ins in bb.instructions:
                si = ins.sync_info
                if si is not None and si.on_wait and len(si.on_wait) > max_waits:
                    waits = list(si.on_wait)
                    for w in waits[:-max_waits]:
                        _waitfix_ctr[0] += 1
                        n = mybir.InstNoOp(
                            name=f"I-waitfix-{_waitfix_ctr[0]}",
                            ins=[], outs=[], engine=ins.engine,
                        )
                        n.sync_info = mybir.SyncInfo(on_wait=[w], on_update=[])
                        out.append(n)
                        total += 1
                    ins.sync_info = mybir.SyncInfo(
                        on_wait=waits[-max_waits:],
                        on_update=list(si.on_update or []),
                    )
                    changed = True
                out.append(ins)
            if changed:
                bb.instructions = out
    return total


def _build():
    nc = bass.Bass()
    xT = nc.declare_dram_parameter("xT", [128, KO, T], BF16, isOutput=False)
    wqk = nc.declare_dram_parameter("wqk", [128, KO, 512], BF16,
                                    isOutput=False)
    wv = nc.declare_dram_parameter("wv", [128, KO, 256], BF16, isOutput=False)
    wo = nc.declare_dram_parameter("wo", [128, 2, D], BF16, isOutput=False)
    cosT = nc.declare_dram_parameter("cosT", [128, T], BF16, isOutput=False)
    sinT = nc.declare_dram_parameter("sinT", [128, T], BF16, isOutput=False)
    seld = nc.declare_dram_parameter("sel", [128, 2, 128], F32R,
                                     isOutput=False)
    # eb[pair, quarter(4), part, ktpos(4), hp, qc, 512]
    ebd = nc.declare_dram_parameter(
        "eb", [2, 4, 128, 4, 2, NQC, 512], BF16, isOutput=False)
    outp = nc.declare_dram_parameter("out", [T, D], F32, isOutput=True)

    with tile.TileContext(nc) as tc:
        with (
            tc.tile_pool(name="const", bufs=1) as cst,
            tc.tile_pool(name="pers", bufs=1) as pers,
            tc.tile_pool(name="ebp", bufs=3) as ebp,
            tc.tile_pool(name="px", bufs=3) as px,
            tc.tile_pool(name="pa", bufs=2) as pa,
            tc.tile_pool(name="pex", bufs=3) as pex,
            tc.tile_pool(name="pbn", bufs=2) as pbn,
            tc.tile_pool(name="pob", bufs=3) as pob,
            tc.tile_pool(name="pap", bufs=2, space="PSUM") as pap,
            tc.tile_pool(name="acc", bufs=1, space="PSUM") as acc,
        ):
            # ---- weights / constants (DMA order matters: SP queue FIFO) ----
            wqk_sb = cst.tile([128, KO, 512], BF16)
            nc.sync.dma_start(wqk_sb[:], wqk[:])
            cos_sb = cst.tile([128, T], BF16)
            nc.sync.dma_start(cos_sb[:], cosT[:])
            sin_sb = cst.tile([128, T], BF16)
            nc.sync.dma_start(sin_sb[:], sinT[:])
            wv_sb = cst.tile([128, KO, 256], BF16)
            nc.sync.dma_start(wv_sb[:], wv[:])

            onesD = cst.tile([128, 1], BF16)
            nc.vector.memset(onesD[:], 1.0)
            sel = cst.tile([128, 2, 128], F32R)
            nc.sync.dma_start(sel[:], seld[:])

            # persistent per-chunk tensors
            qT = [pers.tile([128, 2, 512], BF16, name=f"qT{i}")
                  for i in range(NCH)]
            kT = [pers.tile([128, 2, 512], BF16, name=f"kT{i}")
                  for i in range(NCH)]
            v1 = [pers.tile([128, 4, 256], BF16, name=f"v1{i}")
                  for i in range(NCH)]
            valsT = [pers.tile([128, 2, 512], BF16, name=f"vals{i}")
                     for i in range(NQC)]

            eb_t = {}

            def eb_fetch(g):
                """prefetch eb global-quarter g (= qc*4 + quarter), both
                pairs.  Ring bufs=3/pair -> never queue-blocks x DMAs."""
                if g >= NQC * 4:
                    return
                qc, qtr = divmod(g, 4)
                for pr in range(2):
                    e = ebp.tile([128, 4, 2, 512], BF16, tag=f"eb{pr}",
                                 name=f"eb_{qc}_{pr}_{qtr}")
                    nc.sync.dma_start(e[:], ebd[pr, qtr, :, :, :, qc, :])
                    eb_t[(qc, pr, qtr)] = e

            # x-chunk DMAs interleaved with the first eb quarters
            xc = []
            for ch in range(NCH):
                t = px.tile([128, KO, 512], BF16, tag="xc", name=f"xc{ch}")
                nc.sync.dma_start(t[:], xT[:, :, ch * 512:(ch + 1) * 512])
                xc.append(t)
                if ch < 2:
                    eb_fetch(ch)
            wo_sb = cst.tile([128, 2, D], BF16)
            nc.sync.dma_start(wo_sb[:], wo[:])

            def emit_A_chunk(ch):
                """qkv projection + rope.  rotate_half is a partition swap:
                u = pq*s1 (signed swapped sin), then SBUF->SBUF DMA strip
                copies swap the 32-row halves, then dst = pq*cos + u_sw."""
                cs = ch * 512
                t1b = pa.tile([128, 4, 512], F32, tag="t1")
                ub = pa.tile([128, 4, 512], BF16, tag="ub")
                for pr in range(2):
                    pla = pap.tile([128, 2, 512], F32, tag="plp", bufs=2,
                                   name=f"pla_{ch}_{pr}")
                    for mi in range(2):      # 0 = q, 1 = k
                        blk = (pr * 2 + mi) * 128
                        for ko in range(KO):
                            nc.tensor.matmul(
                                pla[:, mi, :], wqk_sb[:, ko, blk:blk + 128],
                                xc[ch][:, ko], start=(ko == 0),
                                stop=(ko == KO - 1))
                    for mi in range(2):
                        j = pr * 2 + mi
                        nc.vector.tensor_mul(
                            out=t1b[:, j, :], in0=pla[:, mi, :],
                            in1=cos_sb[:, cs:cs + 512])
                        nc.vector.tensor_mul(
                            out=ub[:, j, :], in0=pla[:, mi, :],
                            in1=sin_sb[:, cs:cs + 512])
                usw = pa.tile([128, 4, 512], BF16, tag="usw")
                for b0 in (0, 64):
                    nc.scalar.dma_start(usw[b0:b0 + 32], ub[b0 + 32:b0 + 64])
                    nc.scalar.dma_start(usw[b0 + 32:b0 + 64], ub[b0:b0 + 32])
                for pr in range(2):
                    for mi in range(2):
                        j = pr * 2 + mi
                        dst = qT[ch] if mi == 0 else kT[ch]
                        nc.vector.tensor_add(
                            out=dst[:, pr, :], in0=t1b[:, j, :],
                            in1=usw[:, j, :])
                for tt in range(4):
                    pvw = pap.tile([128, 512], F32, tag="pl", bufs=1,
                                   name=f"pv_{ch}_{tt}")
                    pv = pvw[:, 0:256]
                    for ko in range(KO):
                        nc.tensor.matmul(
                            pv, xc[ch][:, ko, tt * 128:(tt + 1) * 128],
                            wv_sb[:, ko], start=(ko == 0),
                            stop=(ko == KO - 1))
                    nc.vector.tensor_copy(out=v1[ch][:, tt, :], in_=pv)

            # ---------------- attention + o_proj, A interleaved ----------
            # Tails (normalize + o_proj) are emitted AFTER the next q-block's
            # first kt-group so the PE queue never head-of-line blocks on the
            # reciprocal chain (kept HAM re-throttling every qc otherwise).
            SKEW = 1   # kt steps the PV/den matmuls lag behind

            def emit_tail(qc, pvt, den4):
                qs = qc * 512
                rec = pbn.tile([128, 512], F32, tag="rec")
                nc.vector.tensor_copy(out=rec[:], in_=den4[:])
                recf = pbn.tile([128, 512], F32, tag="recf")
                nc.vector.reciprocal(recf[:], rec[:])
                rcast = pbn.tile([128, 512], F32R, tag="rcast")
                nc.vector.tensor_copy(out=rcast[:], in_=recf[:])
                for pr in range(2):
                    bc = pap.tile([128, 512], F32, tag="pl", bufs=1)
                    nc.tensor.matmul(bc[:], sel[:, pr, :], rcast[:],
                                     start=True, stop=True)
                    bcs = pbn.tile([128, 512], F32, tag="bcs")
                    nc.vector.tensor_copy(out=bcs[:], in_=bc[:])
                    nc.vector.tensor_mul(
                        out=valsT[qc][:, pr, :], in0=pvt[pr][:], in1=bcs[:])
                for tt in range(4):
                    for d2 in range(2):
                        po = pap.tile([128, 512], F32, tag="pl", bufs=1)
                        for pr in range(2):
                            nc.tensor.matmul(
                                po[:],
                                valsT[qc][:, pr, tt * 128:(tt + 1) * 128],
                                wo_sb[:, pr, d2 * 512:(d2 + 1) * 512],
                                start=(pr == 0), stop=(pr == 1))
                        ob = pob.tile([128, 512], F32, tag="ob")
                        nc.vector.tensor_copy(out=ob[:], in_=po[:])
                        nc.sync.dma_start(
                            outp[qs + tt * 128:qs + (tt + 1) * 128,
                                 d2 * 512:(d2 + 1) * 512], ob[:])

            prev_tail = None
            for qc in range(NQC):
                pvt = [acc.tile([128, 512], F32, tag=f"pvt{pr}",
                                name=f"pvt_{qc}_{pr}") for pr in range(2)]
                den4 = acc.tile([128, 512], F32, tag="den", name=f"den_{qc}")
                nc.vector.memset(den4[:], 1.0)
                pend = []

                def flush(limit, pvt=pvt, den4=den4, pend=pend):
                    while len(pend) > limit:
                        fkt, fexs = pend.pop(0)
                        fch, tti = fkt // 4, fkt % 4
                        st, sp = (fkt == 0), (fkt == NKT - 1)
                        for fpr in range(2):
                            for hp in range(2):
                                nc.tensor.matmul(
                                    pvt[fpr][hp * 64:(hp + 1) * 64, :],
                                    v1[fch][:, tti,
                                            fpr * 128 + hp * 64:
                                            fpr * 128 + (hp + 1) * 64],
                                    fexs[fpr][:, hp, :], start=st, stop=sp,
                                    skip_group_check=True)
                        for fpr in range(2):
                            for hp in range(2):
                                hh = 2 * fpr + hp
                                nc.tensor.matmul(
                                    den4[32 * hh:32 * hh + 1, :],
                                    onesD[:], fexs[fpr][:, hp, :],
                                    start=st, stop=sp,
                                    skip_group_check=True,
                                    tile_position=(0, 32 * hh))

                for kt in range(NKT):
                    if qc == 0 and (kt == 0 or (kt % 4 == 2 and kt < 12)):
                        # chunk g arrives 2 kt-steps before group g needs it
                        emit_A_chunk(kt // 4 + (1 if kt else 0))
                    if kt % 4 == 0:
                        eb_fetch(qc * 4 + kt // 4 + 2)
                    ch, tti = kt // 4, kt % 4
                    exfs = [None, None]
                    for pr in range(2):
                        pl2 = pap.tile([128, 2, 512], F32, tag="plp",
                                       bufs=2)
                        for hp in range(2):
                            h0 = hp * 64
                            nc.tensor.matmul(
                                pl2[:, hp, :],
                                kT[ch][h0:h0 + 64, pr,
                                       tti * 128:(tti + 1) * 128],
                                qT[qc][h0:h0 + 64, pr, :],
                                start=True, stop=True)
                        ex2 = pex.tile([128, 2, 512], BF16, tag="ex",
                                       bufs=4)
                        nc.scalar.activation(ex2[:], pl2[:], AF.Exp)
                        exf = pex.tile([128, 2, 512], BF16, tag="exf",
                                       bufs=6)
                        mul_eng = nc.vector if pr == 0 else nc.gpsimd
                        mul_eng.tensor_mul(
                            out=exf[:], in0=ex2[:],
                            in1=eb_t[(qc, pr, kt // 4)][:, tti, :, :])
                        exfs[pr] = exf
                    pend.append((kt, exfs))
                    flush(SKEW)
                    if kt == 3 and prev_tail is not None:
                        emit_tail(*prev_tail)
                        prev_tail = None
                flush(0)
                prev_tail = (qc, pvt, den4)
            emit_tail(*prev_tail)

    _split_waits(nc)
    return nc


_nc_cache = None


def _get_nc():
    global _nc_cache
    if _nc_cache is None:
        _nc_cache = _build()
    return _nc_cache


def _prep_inputs(x, pos_bias, sinusoidal_pos, mask, W_qkv, W_o):
    scale = np.float32(1.0 / np.sqrt(HD))
    sp = np.asarray(sinusoidal_pos, np.float32)[0, 0]         # [S, HD]

    cos_t = np.cos(sp).T                                      # [HD, S]
    sin_t = np.sin(sp).T
    cos2_np = np.tile(cos_t, (2, 1)).astype(ml_dtypes.bfloat16)
    # s1: signed swapped sin so that u[p]=pq[p]*s1[p] gives, after the
    # 32-row half swap, u_sw[p] = rotate_half(pq)[p] * sin[p]
    s1_64 = np.concatenate([sin_t[HD // 2:], -sin_t[:HD // 2]], axis=0)
    sin2_np = np.tile(s1_64, (2, 1)).astype(ml_dtypes.bfloat16)

    mask01T = (np.asarray(mask)[0, 0].T != 0)                 # [S(k), S(q)]

    Wh = W_qkv.reshape(H, 3 * HD, D)

    in_maps = []
    for c in range(NCORES):
        b, cg = divmod(c, 4)
        hs = [4 * cg + i for i in range(4)]

        xT_np = np.ascontiguousarray(
            x[b].T.reshape(KO, 128, T).transpose(1, 0, 2)
        ).astype(ml_dtypes.bfloat16)                          # [128, KO, T]

        cols = []
        for pr in range(2):
            h0, h1 = hs[2 * pr], hs[2 * pr + 1]
            for mi in range(2):
                lo, hi = mi * HD, (mi + 1) * HD
                s0 = scale if mi == 0 else np.float32(1.0)
                cols.append(np.concatenate(
                    [Wh[h0, lo:hi] * s0, Wh[h1, lo:hi] * s0], axis=0))
        Wqk_c = np.concatenate(cols, axis=0)                  # [512, D]
        wqk_np = np.ascontiguousarray(
            Wqk_c.T.reshape(KO, 128, 512).transpose(1, 0, 2)
        ).astype(ml_dtypes.bfloat16)

        Wv_c = np.concatenate(
            [Wh[h, 2 * HD:] for h in hs], axis=0)             # [256, D]
        wv_np = np.ascontiguousarray(
            Wv_c.T.reshape(KO, 128, 256).transpose(1, 0, 2)
        ).astype(ml_dtypes.bfloat16)

        wo_np = np.empty((128, 2, D), ml_dtypes.bfloat16)
        for pr in range(2):
            h0, h1 = hs[2 * pr], hs[2 * pr + 1]
            wo_np[0:64, pr, :] = W_o[:, h0 * HD:(h0 + 1) * HD].T
            wo_np[64:128, pr, :] = W_o[:, h1 * HD:(h1 + 1) * HD].T

        # eb[pair, quarter, part, ktpos, hp, qc, 512]
        eb_np = np.empty((2, 4, 128, 4, 2, NQC, 512), ml_dtypes.bfloat16)
        for pr in range(2):
            for hp in range(2):
                h = hs[2 * pr + hp]
                ebT = np.exp(pos_bias[0, h].T * scale)
                ebT = np.where(mask01T, ebT, np.float32(0.0))  # [S(k), S(q)]
                r = ebT.reshape(4, 4, 128, NQC, 512)
                eb_np[pr, :, :, :, hp, :, :] = r.transpose(0, 2, 1, 3, 4)

        # sel[p, pr, m] = 1 iff p == 64*pr + 32*(m//64): broadcasts the
        # reciprocal rows (at partitions 32h) down each pair's 64-row halves
        sel_np = np.zeros((128, 2, 128), np.float32)
        for pr in range(2):
            sel_np[64 * pr, pr, 0:64] = 1.0
            sel_np[64 * pr + 32, pr, 64:128] = 1.0
        in_maps.append({
            "xT": xT_np, "wqk": wqk_np, "wv": wv_np, "wo": wo_np,
            "cosT": cos2_np, "sinT": sin2_np, "sel": sel_np,
            "eb": np.ascontiguousarray(eb_np),
        })
    return in_maps


def _ensure_profile_hook():
    import sys
    import types
    try:
        from antenv.axon_hooks import get_axon_ntff_profile_hook  # noqa
        return
    except ImportError:
        pass
    try:
        from trn_agent_boot.trn_boot import _ntff_profile_via_ctypes
        hook = _ntff_profile_via_ctypes("/opt/axon/libaxon_pjrt.so")
        mod = types.ModuleType("antenv.axon_hooks")
        mod.get_axon_ntff_profile_hook = lambda: hook
        mod.set_axon_ntff_profile_hook = lambda h: None
        sys.modules["antenv.axon_hooks"] = mod
    except Exception:
        pass


def kernel(x, pos_bias, sinusoidal_pos, mask, W_qkv, b_qkv, W_o, b_o):
    global LAST_RESULT
    if TRACE:
        _ensure_profile_hook()
    x = np.asarray(x, np.float32)
    pos_bias = np.asarray(pos_bias, np.float32)
    W_qkv = np.asarray(W_qkv, np.float32)
    W_o = np.asarray(W_o, np.float32)
    b_qkv = np.asarray(b_qkv, np.float32)
    assert not np.any(b_qkv), "nonzero b_qkv not supported by this kernel"
    in_maps = _prep_inputs(x, pos_bias, sinusoidal_pos, mask, W_qkv, W_o)
    nc = _get_nc()
    try:
        r = run_bass_kernel_spmd(nc, in_maps, list(range(NCORES)),
                                 trace=TRACE)
    except Exception:
        r = run_bass_kernel_spmd(nc, in_maps, list(range(NCORES)),
                                 trace=TRACE)
    LAST_RESULT = r
    b_o64 = np.asarray(b_o, np.float32).astype(np.float64)
    out = np.empty((B, S, D), np.float32)
    for b in range(B):
        partial = np.zeros((T, D), np.float64)
        for cg in range(4):
            partial += r.results[4 * b + cg]["out"].astype(np.float64)
        out[b] = (partial + b_o64).astype(np.float32)
    return out
